# revision 2
# baseline (speedup 1.0000x reference)
"""GrwSmoothingLoss on 8 Trainium2 NeuronCores.

Math: with Gram matrix G_b = Z_b @ Z_b^T (8x8) and P_p the permutation
matrix of perm p, the permuted second-difference energy is
  ||diff2(Z_b[perm_p])||^2 = <C_p, G_b>,  C_p = P_p^T (D2^T D2) P_p,
i.e. C_p[i,j] = A[ip_i, ip_j] with A = D2^T D2 and ip the inverse perm.
Z is unit-norm along K, so diag(G_b) == 1 and the diagonal contribution
sum_i A[ip_i, ip_i] = tr(A) = 36 is the same for every p; it cancels in
logsumexp - logit_0.  Only the 28 strictly-upper entries of G matter:
  Xoff[b,p]   = sum_{i<j} 2*A[ip_i, ip_j] * G_b[i,j]          (cmat cols 0..999)
  logits[b,p] = -0.5*(36 + Xoff[b,p])
  V_b         = 7 + sum_{i<j} C1[i,j] * G_b[i,j]
The per-batch loss is ln(sum_p exp(-.5*Xoff)) + 0.5*Xoff[b,0] + a*V_b.
cmat col 1000 folds the last two terms: Xc_b = g_b . (A_up + .5*C1_up).

Device work per core (32 batches): 7 triangular fp16 pair-products +
one k'-reduce (DVE), a one-hot matmul folding the 4-way k-split and the
transpose (PE), the [28]x[28,1008] logits matmul (PE), two Exp+accum
(ACT).  Ships (s1, s2, Xc) per batch; host does ln + mean.

Sharding: data-parallel over B (32 batches/core); cmat/q4 replicated.
"""

import numpy as np

import concourse.bacc as bacc
import concourse.bass as bass
import concourse.mybir as mybir
import concourse.tile as tile
from concourse.bass_utils import run_bass_kernel_spmd

B, T, K = 256, 8, 128
NUM_PERMS = 1000
ALPHA = 0.5
N_CORES = 8
B_LOC = B // N_CORES
NPAIR = T * (T - 1) // 2  # 28
PCOLS = 1008              # 1000 perms + combined col + pad to 8
F32 = mybir.dt.float32
F16 = mybir.dt.float16

_cache = {}

# pair order: (0,1),(0,2),...,(0,7),(1,2),... == np.triu_indices(8, 1)
_IU = np.triu_indices(T, 1)
_OFF = np.concatenate([[0], np.cumsum(np.arange(T - 1, 0, -1))])  # group starts


def _difmat(n, order):
    D = np.eye(T)
    for _ in range(order):
        D = D[1:] - D[:-1]
    return D


_A = _difmat(T, 2).T @ _difmat(T, 2)    # 8x8, second-difference Gram
_C1 = _difmat(T, 1).T @ _difmat(T, 1)   # 8x8, first-difference Gram


def _consts():
    # q4[(b*4+q), b'] = 1 iff b'==b : folds the 4-way k-split reduction and
    # the transpose to [pair, b] into one PE matmul
    q4 = np.repeat(np.eye(B_LOC, dtype=np.float32), 4, axis=0)
    return q4


def _cmat(perm_index):
    perm = np.asarray(perm_index, dtype=np.int64).reshape(NUM_PERMS, T)
    ip = np.empty_like(perm)
    ip[np.arange(NUM_PERMS)[:, None], perm] = np.arange(T)[None, :]
    # Cup[p, pair] = 2*A[ip_i, ip_j] for i<j
    cup = 2.0 * _A[ip[:, _IU[0]], ip[:, _IU[1]]]          # [1000, 28]
    ccomb = 0.5 * cup[0] + ALPHA * _C1[_IU]               # [28]
    cm = np.zeros((NPAIR, PCOLS), dtype=np.float32)
    cm[:, :NUM_PERMS] = cup.T
    cm[:, NUM_PERMS] = ccomb
    return cm.astype(np.float16)


def _kernel_body(tc, out_part, zb_d, cmat_d, q4_d):
    nc = tc.nc
    with (
        tc.tile_pool(name="sb", bufs=1) as sb,
        tc.tile_pool(name="ps", bufs=1, space="PSUM") as ps,
    ):
        zb = sb.tile([128, 256], F16)
        cmat = sb.tile([NPAIR, PCOLS], F16)
        q4 = sb.tile([128, B_LOC], F32)
        nc.sync.dma_start(out=zb[:], in_=zb_d[:])
        nc.scalar.dma_start(out=q4[:], in_=q4_d[:])
        nc.scalar.dma_start(out=cmat[:], in_=cmat_d[:])

        # pair products pp[(b,q), (pair, k')] = Z[b,i,qk']*Z[b,j,qk'],
        # triangular: group i covers pairs (i, i+1..7)
        zv = zb[:].rearrange("p (t k) -> p t k", t=T)
        pp = sb.tile([128, NPAIR * 32], F16)
        ppv = pp[:].rearrange("p (c k) -> p c k", k=32)
        for i in range(T - 1):
            n = T - 1 - i
            nc.vector.tensor_tensor(
                out=ppv[:, _OFF[i] : _OFF[i] + n, :],
                in0=zv[:, i : i + 1, :].broadcast_to([128, n, 32]),
                in1=zv[:, i + 1 : T, :],
                op=mybir.AluOpType.mult,
            )
        # k'-reduce: gq[(b,q), pair]
        gq = sb.tile([128, NPAIR], F32)
        nc.vector.reduce_sum(out=gq[:], in_=ppv, axis=mybir.AxisListType.X)

        # q-sum + transpose: gT[pair, b]
        psum_g = ps.tile([NPAIR, B_LOC], F32)
        nc.tensor.matmul(psum_g[:], gq[:], q4[:])
        gT = sb.tile([NPAIR, B_LOC], F16)
        nc.scalar.copy(gT[:], psum_g[:])

        # X[b, 0:1000] = Xoff logits (unscaled), X[b, 1000] = Xc
        psum_X = ps.tile([B_LOC, 1024], F32)
        nc.tensor.matmul(psum_X[:, 0:512], gT[:], cmat[:, 0:512])
        nc.tensor.matmul(psum_X[:, 512:PCOLS], gT[:], cmat[:, 512:PCOLS])

        # exp(-0.5*Xoff) summed per batch; no recentering needed since
        # |0.5*Xoff| <= 30 stays comfortably inside fp32 exp range.
        out_sb = sb.tile([B_LOC, 4], F32)
        e1 = sb.tile([B_LOC, 512], F32)
        e2 = sb.tile([B_LOC, 512], F32)
        nc.scalar.activation(
            e1[:], psum_X[:, 0:512], mybir.ActivationFunctionType.Exp,
            scale=-0.5, accum_out=out_sb[:, 0:1],
        )
        nc.scalar.activation(
            e2[:, 0:488], psum_X[:, 512:1000], mybir.ActivationFunctionType.Exp,
            scale=-0.5, accum_out=out_sb[:, 1:2],
        )
        nc.vector.tensor_copy(out_sb[:, 2:3], psum_X[:, 1000:1001])
        nc.sync.dma_start(out=out_part[:], in_=out_sb[:])


def _build():
    if "nc" in _cache:
        return _cache["nc"]
    nc = bacc.Bacc(
        "TRN2",
        target_bir_lowering=False,
        debug=False,
        enable_asserts=False,
        num_devices=N_CORES,
    )
    zb_d = nc.dram_tensor("zb", [128, 256], F16, kind="ExternalInput").ap()
    cmat_d = nc.dram_tensor("cmat", [NPAIR, PCOLS], F16, kind="ExternalInput").ap()
    q4_d = nc.dram_tensor("q4", [128, B_LOC], F32, kind="ExternalInput").ap()
    out_d = nc.dram_tensor("out_part", [B_LOC, 4], F32, kind="ExternalOutput").ap()
    with tile.TileContext(nc) as tc:
        _kernel_body(tc, out_d, zb_d, cmat_d, q4_d)
    nc.compile()
    _cache["nc"] = nc
    return nc


def _in_maps(Z, perm_index):
    cm = _cmat(perm_index)
    q4 = _consts()
    Zf = np.asarray(Z, dtype=np.float32).reshape(B, T, 4, 32)
    in_maps = []
    for c in range(N_CORES):
        zb4 = (
            np.ascontiguousarray(
                Zf[c * B_LOC : (c + 1) * B_LOC].transpose(0, 2, 1, 3)
            )
            .reshape(128, 256)
            .astype(np.float16)
        )
        in_maps.append({"zb": zb4, "cmat": cm, "q4": q4})
    return in_maps


def kernel(Z, perm_index, _trace=False):
    nc = _build()
    in_maps = _in_maps(Z, perm_index)
    res = run_bass_kernel_spmd(
        nc, in_maps, core_ids=list(range(N_CORES)), trace=_trace
    )
    total = np.float64(0.0)
    for r in res.results:
        o = np.asarray(r["out_part"], dtype=np.float64)
        total += np.sum(np.log(o[:, 0] + o[:, 1]) + o[:, 2])
    out = np.array(total / B + ALPHA * (T - 1), dtype=np.float32)
    if _trace:
        return out, res
    return out
